# revision 34
# baseline (speedup 1.0000x reference)
"""Banded multi-head attention (window +-64) for trn2, 8 NeuronCores.

Sharding: batch (2) x sequence blocks (4) -> 8 cores, no collectives.
Each core handles one 512-query block of one batch for ALL 16 heads:
  - loads x^T slices (with a 64-row K/V halo) and full projection weights
  - computes q^T/k^T (dk on partitions), V (seq on partitions)
  - banded scores per (head, 128-query chunk): [128q, 256t] tiles
  - softmax via exp (no max-sub needed: |s/8| < ~4), multiplicative band mask
  - P^T via PE transpose -> PV -> x_att^T -> output projection
Returns (out [2,2048,1024], attn [2,16,2048,2048]); attn is exactly zero
off-band, so the host scatters the compact [128,256] tiles into zeros.

Matmul operands are bf16 (host-cast); accumulation is f32 in PSUM.
Attention probabilities are emitted bf16 and upcast on the host.
"""
import os
import numpy as np
import ml_dtypes
from contextlib import ExitStack

import concourse.bacc as bacc
import concourse.tile as tile
import concourse.mybir as mybir
from concourse import bass_utils

# NTFF tracing under axon needs antenv.axon_hooks, absent in slim agent
# containers; disable tracing only in that case so a stray BASS_TRACE=1
# can't crash the run (native-NRT and hook-equipped axon envs unaffected).
try:
    from concourse._compat import axon_active as _axon_active
    if _axon_active():
        try:
            from antenv import axon_hooks as _axon_hooks  # noqa: F401
        except ImportError:
            os.environ["BASS_NEVER_TRACE"] = "1"
except Exception:
    pass

F32 = mybir.dt.float32
BF16 = mybir.dt.bfloat16
AF = mybir.ActivationFunctionType
ALU = mybir.AluOpType

B, S, D, H, DK = 2, 2048, 1024, 16, 64
NBLK = 4              # sequence blocks per batch
SEQ = S // NBLK       # 512 queries per core
HALO = 64
KV = SEQ + 2 * HALO   # 640 key/value rows per core (zero-padded at edges)
NCH = SEQ // 128      # 4 query chunks of 128
G = D // 128          # 8 contraction chunks
NTB = KV // 128       # 5 V row chunks
SCALE = 1.0 / 8.0     # 1/sqrt(DK)

_NC = None
LAST_RESULTS = None

# tuning knobs (read at build time)
KNOB_WORK_BUFS = int(os.environ.get("K_WORK_BUFS", "4"))
KNOB_PTB = os.environ.get("K_PTB", "alt")      # alt | dve | act
KNOB_XAT = os.environ.get("K_XAT", "act")      # act | dve


def _build():
    nc = bacc.Bacc("TRN2", target_bir_lowering=False, debug=False, num_devices=8)

    xqT_d = nc.dram_tensor("xqT", [D, SEQ], BF16, kind="ExternalInput").ap()
    xkT_d = nc.dram_tensor("xkT", [D, KV], BF16, kind="ExternalInput").ap()
    xvT_d = nc.dram_tensor("xvT", [D, KV], BF16, kind="ExternalInput").ap()
    wq_d = nc.dram_tensor("Wq", [D, D], BF16, kind="ExternalInput").ap()
    wk_d = nc.dram_tensor("Wk", [D, D], BF16, kind="ExternalInput").ap()
    wv_d = nc.dram_tensor("Wv", [D, D], BF16, kind="ExternalInput").ap()
    wo_d = nc.dram_tensor("Wo", [D, D], BF16, kind="ExternalInput").ap()
    b3_d = nc.dram_tensor("b3", [3, D], F32, kind="ExternalInput").ap()
    mask_d = nc.dram_tensor("mask", [NCH, 128, 256], BF16, kind="ExternalInput").ap()
    id_d = nc.dram_tensor("ident", [128, 128], BF16, kind="ExternalInput").ap()

    attn_d = nc.dram_tensor("attn_c", [H, NCH, 128, 256], BF16, kind="ExternalOutput").ap()
    out_d = nc.dram_tensor("out_d", [SEQ, D], F32, kind="ExternalOutput").ap()

    with tile.TileContext(nc) as tc, ExitStack() as ctx:
        persist = ctx.enter_context(tc.tile_pool(name="persist", bufs=1))
        work = ctx.enter_context(tc.tile_pool(name="work", bufs=KNOB_WORK_BUFS))
        ppool = ctx.enter_context(tc.tile_pool(name="pp", bufs=int(os.environ.get("K_PP","2")), space="PSUM"))
        spool = ctx.enter_context(tc.tile_pool(name="ps", bufs=int(os.environ.get("K_PS","3")), space="PSUM"))
        tpool = ctx.enter_context(tc.tile_pool(name="pt", bufs=int(os.environ.get("K_PT","2")), space="PSUM"))
        vpool = ctx.enter_context(tc.tile_pool(name="pv", bufs=int(os.environ.get("K_PV","1")), space="PSUM"))

        # persistent SBUF tensors; [128, G*N] = logical [D, N] in 128-row chunks
        xq_t = persist.tile([128, G * SEQ], BF16)
        xk_t = persist.tile([128, G * KV], BF16)
        xv_t = persist.tile([128, G * KV], BF16)
        wq_t = persist.tile([128, G * D], BF16)
        wk_t = persist.tile([128, G * D], BF16)
        wv_t = persist.tile([128, G * D], BF16)
        wo_t = persist.tile([128, G * D], BF16)
        qT = persist.tile([128, G * SEQ], BF16)   # [dk, q] 2 heads per chunk
        kT = persist.tile([128, G * KV], BF16)    # [dk, t]
        vS = persist.tile([128, NTB * D], BF16)   # [t, dv] per 128-row chunk
        xaT = persist.tile([128, G * SEQ], BF16)  # [dv, q]
        mk_t = persist.tile([128, NCH * 256], BF16)
        id_t = persist.tile([128, 128], BF16)
        b3_t = persist.tile([128, 3 * G], F32)
        ob1_t = persist.tile([128, 8 * 512], F32)  # out-proj first-half sums (c,n)

        def bias(k, m):  # k: 0=bq 1=bk 2=bv, m: dout chunk
            return b3_t[:, k * G + m: k * G + m + 1]

        # -------- input DMAs (batched; issued from multiple idle engines in
        # parallel -- a single sequencer takes ~0.65us per dma_start, which
        # would serialize the whole input load behind ~25us of issue time)
        def load_chunked(dst, src, width, g0, g1, eng=None):
            (eng or nc.sync).dma_start(
                dst[:, g0 * width:g1 * width],
                src[g0 * 128:g1 * 128, :].rearrange("(g p) n -> p g n", p=128),
            )

        # Q-projection prerequisites first, in ~0.5-1MB chunks so they spread
        # across the HW queues and the first matmuls start early.
        for g in range(8):
            load_chunked(wq_t, wq_d, D, g, g + 1, nc.sync)
        for g0, g1 in ((0, 2), (2, 4), (4, 6), (6, 8)):
            load_chunked(xq_t, xqT_d, SEQ, g0, g1, nc.scalar)
        for g0, g1 in ((0, 2), (2, 4), (4, 6), (6, 8)):
            load_chunked(wk_t, wk_d, D, g0, g1, nc.scalar)
        for g0, g1 in ((0, 2), (2, 4), (4, 6), (6, 8)):
            load_chunked(xk_t, xkT_d, KV, g0, g1, nc.gpsimd)
        for g0, g1 in ((0, 2), (2, 4), (4, 6), (6, 8)):
            load_chunked(wv_t, wv_d, D, g0, g1, nc.sync)
        for g0, g1 in ((0, 2), (2, 4), (4, 6), (6, 8)):
            load_chunked(xv_t, xvT_d, KV, g0, g1, nc.sync)
        nc.gpsimd.dma_start(mk_t[:], mask_d.rearrange("c r m -> r c m"))
        nc.gpsimd.dma_start(id_t[:], id_d)
        nc.gpsimd.dma_start(b3_t[:], b3_d.rearrange("k (m p) -> p k m", p=128))
        for g0, g1 in ((0, 2), (2, 4), (4, 6), (6, 8)):
            load_chunked(wo_t, wo_d, D, g0, g1, nc.sync)

        # -------- projections (emitted interleaved with attention pairs below)
        def proj_q(m):
            pq = ppool.tile([128, SEQ], F32, tag="pp")
            for g in range(G):
                nc.tensor.matmul(
                    pq[:],
                    wq_t[:, g * D + m * 128: g * D + (m + 1) * 128],
                    xq_t[:, g * SEQ:(g + 1) * SEQ],
                    start=(g == 0), stop=(g == G - 1),
                )
            nc.scalar.activation(qT[:, m * SEQ:(m + 1) * SEQ], pq[:],
                                 AF.Identity, bias=bias(0, m), scale=1.0)

        def proj_k(m):
            for c0, w in ((0, 512), (512, 128)):
                pk = ppool.tile([128, w], F32, tag="pp")
                for g in range(G):
                    nc.tensor.matmul(
                        pk[:],
                        wk_t[:, g * D + m * 128: g * D + (m + 1) * 128],
                        xk_t[:, g * KV + c0: g * KV + c0 + w],
                        start=(g == 0), stop=(g == G - 1),
                    )
                if w == 512:
                    nc.scalar.activation(kT[:, m * KV + c0: m * KV + c0 + w], pk[:],
                                         AF.Identity, bias=bias(1, m), scale=1.0)
                else:
                    # narrow pass on DVE to offload ACT
                    nc.vector.tensor_scalar_add(
                        kT[:, m * KV + c0: m * KV + c0 + w], pk[:], bias(1, m))

        def proj_v(tb):
            for n in range(2):
                pvp = ppool.tile([128, 512], F32, tag="pp")
                for g in range(G):
                    nc.tensor.matmul(
                        pvp[:],
                        xv_t[:, g * KV + tb * 128: g * KV + (tb + 1) * 128],
                        wv_t[:, g * D + n * 512: g * D + (n + 1) * 512],
                        start=(g == 0), stop=(g == G - 1),
                    )
                if os.environ.get("K_VC", "dve") == "split" and n == 1:
                    nc.scalar.copy(vS[:, tb * D + n * 512: tb * D + (n + 1) * 512], pvp[:])
                else:
                    nc.vector.tensor_copy(vS[:, tb * D + n * 512: tb * D + (n + 1) * 512], pvp[:])

        # -------- attention for one head pair (h0 = 2*hp, h1 = 2*hp+1)
        # HW constraint: each matmul/transpose accumulation group owns a fresh
        # PSUM tile and writes it from offset 0 / partition 0 (sub-bank offsets
        # silently corrupt or crash).
        # fillers: per-chunk emission hooks that keep PE fed with projection
        # matmuls while the softmax chain runs on ACT/DVE/POOL
        def attn_pair(hp, fillers=()):
            m = hp
            attb = work.tile([128, NCH * 512], BF16, tag="attb")
            for c in range(NCH):
                pss = []
                for hh in range(2):
                    psh = spool.tile([128, 256], F32, tag="ps", name=f"ps{hp}_{c}_{hh}")
                    nc.tensor.matmul(
                        psh[:],
                        qT[hh * 64:hh * 64 + 64, m * SEQ + c * 128: m * SEQ + (c + 1) * 128],
                        kT[hh * 64:hh * 64 + 64, m * KV + c * 128: m * KV + c * 128 + 256],
                        start=True, stop=True,
                    )
                    pss.append(psh)
                if c < len(fillers):
                    for f in fillers[c]:
                        f()
                den = work.tile([128, 2], F32, tag="den")
                pms = []
                for hh in range(2):
                    peh = work.tile([128, 256], BF16, tag=f"pe{hh}")
                    nc.scalar.activation(peh[:], pss[hh][:], AF.Exp, scale=SCALE)
                    pm = work.tile([128, 256], BF16, tag=f"pm{hh}")
                    eng = nc.vector
                    eng.scalar_tensor_tensor(
                        pm[:], peh[:], 1.0, mk_t[:, c * 256:(c + 1) * 256],
                        op0=ALU.mult, op1=ALU.mult, accum_out=den[:, hh:hh + 1],
                    )
                    pms.append(pm)
                r2 = work.tile([128, 2], F32, tag="r2")
                nc.vector.reciprocal(r2[:], den[:])
                for hh in range(2):
                    eng = nc.gpsimd if (os.environ.get("K_ATTB", "gps") == "gps" or hh == 0) else nc.vector
                    eng.tensor_scalar_mul(
                        attb[:, c * 512 + hh * 256: c * 512 + (hh + 1) * 256],
                        pms[hh][:], r2[:, hh:hh + 1],
                    )
                for hh in range(2):
                    pvh = vpool.tile([64, 128], F32, tag="pv", name=f"pv{hp}_{c}_{hh}")
                    for b2 in range(2):
                        ptp = tpool.tile([128, 128], BF16, tag="pt", name=f"pt{hp}_{c}_{hh}_{b2}")
                        nc.tensor.transpose(
                            ptp[:],
                            attb[:, c * 512 + hh * 256 + b2 * 128: c * 512 + hh * 256 + (b2 + 1) * 128],
                            id_t[:],
                        )
                        ptb = work.tile([128, 128], BF16, tag=f"ptb{hh}{b2}")
                        kp = os.environ.get("K_PTBC", "d3")
                        use_dve = (kp == "dve" or (kp == "alt" and (hh + b2) % 2 == 0)
                                   or (kp == "d3" and not (hh == 1 and b2 == 1)))
                        if use_dve:
                            nc.vector.tensor_copy(ptb[:], ptp[:])
                        else:
                            nc.scalar.copy(ptb[:], ptp[:])
                        nc.tensor.matmul(
                            pvh[:],
                            vS[:, (c + b2) * D + (2 * hp + hh) * 64: (c + b2) * D + (2 * hp + hh) * 64 + 64],
                            ptb[:],
                            start=(b2 == 0), stop=(b2 == 1),
                        )
                    if hh == 0:
                        nc.scalar.activation(
                            xaT[hh * 64:hh * 64 + 64, m * SEQ + c * 128: m * SEQ + (c + 1) * 128],
                            pvh[:], AF.Identity,
                            bias=bias(2, m)[hh * 64:hh * 64 + 64, :], scale=1.0,
                        )
                    else:
                        nc.vector.tensor_scalar_add(
                            xaT[hh * 64:hh * 64 + 64, m * SEQ + c * 128: m * SEQ + (c + 1) * 128],
                            pvh[:], bias(2, m)[hh * 64:hh * 64 + 64, :])
            for hh in range(2):
                nc.sync.dma_start(
                    attn_d[2 * hp + hh].rearrange("c r m -> r c m"),
                    attb[:].rearrange("p (c h m) -> p c h m", c=NCH, h=2)[:, :, hh, :],
                )

        # -------- emission schedule: feed PE projections early, attention follows
        phase = globals().get("_PHASE", "full")
        if phase != "dma":
            if phase == "proj":
                for m in range(G):
                    proj_q(m); proj_k(m)
                for tb in range(NTB):
                    proj_v(tb)
            else:
                proj_q(0); proj_k(0)
                attn_pair(0, fillers=[
                    [lambda: proj_v(0), lambda: proj_v(1)],
                    [lambda: proj_v(2)],
                    [lambda: proj_v(3)],
                    [lambda: proj_v(4)],
                ])
                npair = {"attn2": 2}.get(phase, G)
                if npair > 1:
                    proj_q(1); proj_k(1)
                def oproj_half1(c_lo=0, c_hi=NCH):
                    if phase != "full":
                        return
                    for c in range(c_lo, c_hi):
                        for n in range(2):
                            po = ppool.tile([128, 512], F32, tag="pp",
                                            name=f"po1_{c}_{n}")
                            for g in range(4):
                                nc.tensor.matmul(
                                    po[:],
                                    xaT[:, g * SEQ + c * 128: g * SEQ + (c + 1) * 128],
                                    wo_t[:, g * D + n * 512: g * D + (n + 1) * 512],
                                    start=(g == 0), stop=(g == 3),
                                )
                            nc.vector.tensor_copy(
                                ob1_t[:, (c * 2 + n) * 512:(c * 2 + n + 1) * 512], po[:])

                for m in range(1, npair):
                    nm = m + 1
                    if nm < npair:
                        fl = [[lambda nm=nm: proj_q(nm)], [lambda nm=nm: proj_k(nm)]]
                    else:
                        fl = []
                    attn_pair(m, fillers=fl)
                    ksplit = os.environ.get("K_OPJ", "6")
                    if ksplit == "56":
                        if m == 5:
                            oproj_half1(0, 2)
                        elif m == 6:
                            oproj_half1(2, 4)
                    else:
                        if m == 6:
                            oproj_half1()

        # -------- output projection, second half (g=4..7) + add of first half
        for c in range(NCH if phase == "full" else 0):
            ob = work.tile([128, D], F32, tag="ob")
            for n in range(2):
                po = ppool.tile([128, 512], F32, tag="pp")
                for g in range(4, G):
                    nc.tensor.matmul(
                        po[:],
                        xaT[:, g * SEQ + c * 128: g * SEQ + (c + 1) * 128],
                        wo_t[:, g * D + n * 512: g * D + (n + 1) * 512],
                        start=(g == 4), stop=(g == G - 1),
                    )
                nc.vector.scalar_tensor_tensor(
                    ob[:, n * 512:(n + 1) * 512], po[:], 0.0,
                    ob1_t[:, (c * 2 + n) * 512:(c * 2 + n + 1) * 512],
                    op0=ALU.add, op1=ALU.add,
                )
                nc.sync.dma_start(
                    out_d[c * 128:(c + 1) * 128, n * 512:(n + 1) * 512],
                    ob[:, n * 512:(n + 1) * 512])

    nc.compile()
    return nc


def _shard_inputs(query, key, value, Wq, bq, Wk, bk, Wv, bv, Wo):
    bf = ml_dtypes.bfloat16
    wq = np.asarray(Wq, np.float32).astype(bf)
    wk = np.asarray(Wk, np.float32).astype(bf)
    wv = np.asarray(Wv, np.float32).astype(bf)
    wo = np.asarray(Wo, np.float32).astype(bf)
    b3 = np.stack([np.asarray(bq, np.float32), np.asarray(bk, np.float32),
                   np.asarray(bv, np.float32)])
    ident = np.eye(128, dtype=bf)

    r_ = np.arange(128)[:, None]
    m_ = np.arange(256)[None, :]
    band = (m_ - r_ >= 0) & (m_ - r_ <= 2 * HALO)

    in_maps = []
    for core in range(8):
        b, blk = divmod(core, NBLK)
        q0 = blk * SEQ
        lo, hi = q0 - HALO, q0 + SEQ + HALO
        s0, s1 = max(lo, 0), min(hi, S)
        xq = np.asarray(query[b, q0:q0 + SEQ, :], np.float32).T.astype(bf)
        xk = np.zeros((D, KV), bf)
        xv = np.zeros((D, KV), bf)
        xk[:, s0 - lo:s1 - lo] = np.asarray(key[b, s0:s1, :], np.float32).T.astype(bf)
        xv[:, s0 - lo:s1 - lo] = np.asarray(value[b, s0:s1, :], np.float32).T.astype(bf)
        mask = np.zeros((NCH, 128, 256), np.float32)
        for c in range(NCH):
            j = lo + c * 128 + m_
            mask[c] = (band & (j >= 0) & (j < S)).astype(np.float32)
        mask = mask.astype(bf)
        in_maps.append({
            "xqT": xq, "xkT": xk, "xvT": xv,
            "Wq": wq, "Wk": wk, "Wv": wv, "Wo": wo,
            "b3": b3, "mask": mask, "ident": ident,
        })
    return in_maps


def kernel(query, key, value, Wq, bq, Wk, bk, Wv, bv, Wo, bo):
    global _NC, LAST_RESULTS
    if _NC is None:
        _NC = _build()
    in_maps = _shard_inputs(query, key, value, Wq, bq, Wk, bk, Wv, bv, Wo)
    res = bass_utils.run_bass_kernel_spmd(_NC, in_maps, core_ids=list(range(8)))
    LAST_RESULTS = res

    bo32 = np.asarray(bo, np.float32)
    out = np.empty((B, S, D), np.float32)
    attn = np.zeros((B, H, S, S), np.float32)
    for core in range(8):
        rr = res.results[core]
        b, blk = divmod(core, NBLK)
        q0 = blk * SEQ
        out[b, q0:q0 + SEQ, :] = rr["out_d"] + bo32[None, :]
        ac = np.asarray(rr["attn_c"]).astype(np.float32)  # [H, NCH, 128, 256]
        lo = q0 - HALO
        for c in range(NCH):
            j0 = lo + c * 128
            jlo, jhi = max(j0, 0), min(j0 + 256, S)
            attn[b, :, q0 + c * 128: q0 + (c + 1) * 128, jlo:jhi] = \
                ac[:, c, :, jlo - j0:jhi - j0]
    return out, attn



# revision 35
# speedup vs baseline: 1.0103x; 1.0103x over previous
"""Banded multi-head attention (window +-64) for trn2, 8 NeuronCores.

Sharding: batch (2) x sequence blocks (4) -> 8 cores, no collectives.
Each core handles one 512-query block of one batch for ALL 16 heads:
  - loads x^T slices (with a 64-row K/V halo) and full projection weights
  - computes q^T/k^T (dk on partitions), V (seq on partitions)
  - banded scores per (head, 128-query chunk): [128q, 256t] tiles
  - softmax via exp (no max-sub needed: |s/8| < ~4), multiplicative band mask
  - P^T via PE transpose -> PV -> x_att^T -> output projection
Returns (out [2,2048,1024], attn [2,16,2048,2048]); attn is exactly zero
off-band, so the host scatters the compact [128,256] tiles into zeros.

Matmul operands are bf16 (host-cast); accumulation is f32 in PSUM.
Attention probabilities are emitted bf16 and upcast on the host.
"""
import os
import numpy as np
import ml_dtypes
from contextlib import ExitStack

import concourse.bacc as bacc
import concourse.tile as tile
import concourse.mybir as mybir
from concourse import bass_utils

# NTFF tracing under axon needs antenv.axon_hooks, absent in slim agent
# containers; disable tracing only in that case so a stray BASS_TRACE=1
# can't crash the run (native-NRT and hook-equipped axon envs unaffected).
try:
    from concourse._compat import axon_active as _axon_active
    if _axon_active():
        try:
            from antenv import axon_hooks as _axon_hooks  # noqa: F401
        except ImportError:
            os.environ["BASS_NEVER_TRACE"] = "1"
except Exception:
    pass

F32 = mybir.dt.float32
BF16 = mybir.dt.bfloat16
AF = mybir.ActivationFunctionType
ALU = mybir.AluOpType

B, S, D, H, DK = 2, 2048, 1024, 16, 64
NBLK = 4              # sequence blocks per batch
SEQ = S // NBLK       # 512 queries per core
HALO = 64
KV = SEQ + 2 * HALO   # 640 key/value rows per core (zero-padded at edges)
NCH = SEQ // 128      # 4 query chunks of 128
G = D // 128          # 8 contraction chunks
NTB = KV // 128       # 5 V row chunks
SCALE = 1.0 / 8.0     # 1/sqrt(DK)

_NC = None
LAST_RESULTS = None

# tuning knobs (read at build time)
KNOB_WORK_BUFS = int(os.environ.get("K_WORK_BUFS", "4"))
KNOB_PTB = os.environ.get("K_PTB", "alt")      # alt | dve | act
KNOB_XAT = os.environ.get("K_XAT", "act")      # act | dve


def _build():
    nc = bacc.Bacc("TRN2", target_bir_lowering=False, debug=False, num_devices=8)

    xqT_d = nc.dram_tensor("xqT", [D, SEQ], BF16, kind="ExternalInput").ap()
    xkT_d = nc.dram_tensor("xkT", [D, KV], BF16, kind="ExternalInput").ap()
    xvT_d = nc.dram_tensor("xvT", [D, KV], BF16, kind="ExternalInput").ap()
    wq_d = nc.dram_tensor("Wq", [D, D], BF16, kind="ExternalInput").ap()
    wk_d = nc.dram_tensor("Wk", [D, D], BF16, kind="ExternalInput").ap()
    wv_d = nc.dram_tensor("Wv", [D, D], BF16, kind="ExternalInput").ap()
    wo_d = nc.dram_tensor("Wo", [D, D], BF16, kind="ExternalInput").ap()
    b3_d = nc.dram_tensor("b3", [3, D], F32, kind="ExternalInput").ap()
    mask_d = nc.dram_tensor("mask", [NCH, 128, 256], BF16, kind="ExternalInput").ap()
    id_d = nc.dram_tensor("ident", [128, 128], BF16, kind="ExternalInput").ap()

    attn_d = nc.dram_tensor("attn_c", [H, NCH, 128, 256], BF16, kind="ExternalOutput").ap()
    out_d = nc.dram_tensor("out_d", [SEQ, D], F32, kind="ExternalOutput").ap()

    with tile.TileContext(nc) as tc, ExitStack() as ctx:
        persist = ctx.enter_context(tc.tile_pool(name="persist", bufs=1))
        work = ctx.enter_context(tc.tile_pool(name="work", bufs=KNOB_WORK_BUFS))
        ppool = ctx.enter_context(tc.tile_pool(name="pp", bufs=int(os.environ.get("K_PP","2")), space="PSUM"))
        spool = ctx.enter_context(tc.tile_pool(name="ps", bufs=int(os.environ.get("K_PS","3")), space="PSUM"))
        tpool = ctx.enter_context(tc.tile_pool(name="pt", bufs=int(os.environ.get("K_PT","2")), space="PSUM"))
        vpool = ctx.enter_context(tc.tile_pool(name="pv", bufs=int(os.environ.get("K_PV","1")), space="PSUM"))

        # persistent SBUF tensors; [128, G*N] = logical [D, N] in 128-row chunks
        xq_t = persist.tile([128, G * SEQ], BF16)
        xk_t = persist.tile([128, G * KV], BF16)
        xv_t = persist.tile([128, G * KV], BF16)
        wq_t = persist.tile([128, G * D], BF16)
        wk_t = persist.tile([128, G * D], BF16)
        wv_t = persist.tile([128, G * D], BF16)
        wo_t = persist.tile([128, G * D], BF16)
        qT = persist.tile([128, G * SEQ], BF16)   # [dk, q] 2 heads per chunk
        kT = persist.tile([128, G * KV], BF16)    # [dk, t]
        vS = persist.tile([128, NTB * D], BF16)   # [t, dv] per 128-row chunk
        xaT = persist.tile([128, G * SEQ], BF16)  # [dv, q]
        mk_t = persist.tile([128, NCH * 256], BF16)
        id_t = persist.tile([128, 128], BF16)
        b3_t = persist.tile([128, 3 * G], F32)
        ob1_t = persist.tile([128, 8 * 512], F32)  # out-proj first-half sums (c,n)

        def bias(k, m):  # k: 0=bq 1=bk 2=bv, m: dout chunk
            return b3_t[:, k * G + m: k * G + m + 1]

        # -------- input DMAs (batched; issued from multiple idle engines in
        # parallel -- a single sequencer takes ~0.65us per dma_start, which
        # would serialize the whole input load behind ~25us of issue time)
        def load_chunked(dst, src, width, g0, g1, eng=None):
            (eng or nc.sync).dma_start(
                dst[:, g0 * width:g1 * width],
                src[g0 * 128:g1 * 128, :].rearrange("(g p) n -> p g n", p=128),
            )

        # Q-projection prerequisites first, in ~0.5-1MB chunks so they spread
        # across the HW queues and the first matmuls start early.
        for g in range(8):
            load_chunked(wq_t, wq_d, D, g, g + 1, nc.sync)
        for g0, g1 in ((0, 2), (2, 4), (4, 6), (6, 8)):
            load_chunked(xq_t, xqT_d, SEQ, g0, g1, nc.scalar)
        for g0, g1 in ((0, 2), (2, 4), (4, 6), (6, 8)):
            load_chunked(wk_t, wk_d, D, g0, g1, nc.scalar)
        for g0, g1 in ((0, 2), (2, 4), (4, 6), (6, 8)):
            load_chunked(xk_t, xkT_d, KV, g0, g1, nc.gpsimd)
        for g0, g1 in ((0, 2), (2, 4), (4, 6), (6, 8)):
            load_chunked(wv_t, wv_d, D, g0, g1, nc.sync)
        for g0, g1 in ((0, 2), (2, 4), (4, 6), (6, 8)):
            load_chunked(xv_t, xvT_d, KV, g0, g1, nc.sync)
        nc.gpsimd.dma_start(mk_t[:], mask_d.rearrange("c r m -> r c m"))
        nc.gpsimd.dma_start(id_t[:], id_d)
        nc.gpsimd.dma_start(b3_t[:], b3_d.rearrange("k (m p) -> p k m", p=128))
        for g0, g1 in ((0, 2), (2, 4), (4, 6), (6, 8)):
            load_chunked(wo_t, wo_d, D, g0, g1, nc.sync)

        # -------- projections (emitted interleaved with attention pairs below)
        def proj_q(m):
            pq = ppool.tile([128, SEQ], F32, tag="pp")
            for g in range(G):
                nc.tensor.matmul(
                    pq[:],
                    wq_t[:, g * D + m * 128: g * D + (m + 1) * 128],
                    xq_t[:, g * SEQ:(g + 1) * SEQ],
                    start=(g == 0), stop=(g == G - 1),
                )
            nc.scalar.activation(qT[:, m * SEQ:(m + 1) * SEQ], pq[:],
                                 AF.Identity, bias=bias(0, m), scale=1.0)

        def proj_k(m):
            for c0, w in ((0, 512), (512, 128)):
                pk = ppool.tile([128, w], F32, tag="pp")
                for g in range(G):
                    nc.tensor.matmul(
                        pk[:],
                        wk_t[:, g * D + m * 128: g * D + (m + 1) * 128],
                        xk_t[:, g * KV + c0: g * KV + c0 + w],
                        start=(g == 0), stop=(g == G - 1),
                    )
                if w == 512:
                    nc.scalar.activation(kT[:, m * KV + c0: m * KV + c0 + w], pk[:],
                                         AF.Identity, bias=bias(1, m), scale=1.0)
                else:
                    # narrow pass on DVE to offload ACT
                    nc.vector.tensor_scalar_add(
                        kT[:, m * KV + c0: m * KV + c0 + w], pk[:], bias(1, m))

        def proj_v(tb):
            for n in range(2):
                pvp = ppool.tile([128, 512], F32, tag="pp")
                for g in range(G):
                    nc.tensor.matmul(
                        pvp[:],
                        xv_t[:, g * KV + tb * 128: g * KV + (tb + 1) * 128],
                        wv_t[:, g * D + n * 512: g * D + (n + 1) * 512],
                        start=(g == 0), stop=(g == G - 1),
                    )
                if os.environ.get("K_VC", "dve") == "split" and n == 1:
                    nc.scalar.copy(vS[:, tb * D + n * 512: tb * D + (n + 1) * 512], pvp[:])
                else:
                    nc.vector.tensor_copy(vS[:, tb * D + n * 512: tb * D + (n + 1) * 512], pvp[:])

        # -------- attention for one head pair (h0 = 2*hp, h1 = 2*hp+1)
        # HW constraint: each matmul/transpose accumulation group owns a fresh
        # PSUM tile and writes it from offset 0 / partition 0 (sub-bank offsets
        # silently corrupt or crash).
        # fillers: per-chunk emission hooks that keep PE fed with projection
        # matmuls while the softmax chain runs on ACT/DVE/POOL
        def attn_pair(hp, fillers=()):
            m = hp
            attb = work.tile([128, NCH * 512], BF16, tag="attb")
            for c in range(NCH):
                pss = []
                for hh in range(2):
                    psh = spool.tile([128, 256], F32, tag="ps", name=f"ps{hp}_{c}_{hh}")
                    nc.tensor.matmul(
                        psh[:],
                        qT[hh * 64:hh * 64 + 64, m * SEQ + c * 128: m * SEQ + (c + 1) * 128],
                        kT[hh * 64:hh * 64 + 64, m * KV + c * 128: m * KV + c * 128 + 256],
                        start=True, stop=True,
                    )
                    pss.append(psh)
                if c < len(fillers):
                    for f in fillers[c]:
                        f()
                pms, rs = [], []
                for hh in range(2):
                    peh = work.tile([128, 256], BF16, tag=f"pe{hh}")
                    nc.scalar.activation(peh[:], pss[hh][:], AF.Exp, scale=SCALE)
                    pm = work.tile([128, 256], BF16, tag=f"pm{hh}")
                    den = work.tile([128, 1], F32, tag=f"den{hh}", name=f"den{hp}_{c}_{hh}")
                    nc.vector.scalar_tensor_tensor(
                        pm[:], peh[:], 1.0, mk_t[:, c * 256:(c + 1) * 256],
                        op0=ALU.mult, op1=ALU.mult, accum_out=den[:],
                    )
                    rh = work.tile([128, 1], F32, tag=f"r{hh}", name=f"r{hp}_{c}_{hh}")
                    nc.vector.reciprocal(rh[:], den[:])
                    nc.gpsimd.tensor_scalar_mul(
                        attb[:, c * 512 + hh * 256: c * 512 + (hh + 1) * 256],
                        pm[:], rh[:],
                    )
                    pms.append(pm)
                    rs.append(rh)
                for hh in range(2):
                    pvh = vpool.tile([64, 128], F32, tag="pv", name=f"pv{hp}_{c}_{hh}")
                    for b2 in range(2):
                        ptp = tpool.tile([128, 128], BF16, tag="pt", name=f"pt{hp}_{c}_{hh}_{b2}")
                        nc.tensor.transpose(
                            ptp[:],
                            attb[:, c * 512 + hh * 256 + b2 * 128: c * 512 + hh * 256 + (b2 + 1) * 128],
                            id_t[:],
                        )
                        ptb = work.tile([128, 128], BF16, tag=f"ptb{hh}{b2}")
                        kp = os.environ.get("K_PTBC", "d3")
                        use_dve = (kp == "dve" or (kp == "alt" and (hh + b2) % 2 == 0)
                                   or (kp == "d3" and not (hh == 1 and b2 == 1)))
                        if use_dve:
                            nc.vector.tensor_copy(ptb[:], ptp[:])
                        else:
                            nc.scalar.copy(ptb[:], ptp[:])
                        nc.tensor.matmul(
                            pvh[:],
                            vS[:, (c + b2) * D + (2 * hp + hh) * 64: (c + b2) * D + (2 * hp + hh) * 64 + 64],
                            ptb[:],
                            start=(b2 == 0), stop=(b2 == 1),
                        )
                    if hh == 0:
                        nc.scalar.activation(
                            xaT[hh * 64:hh * 64 + 64, m * SEQ + c * 128: m * SEQ + (c + 1) * 128],
                            pvh[:], AF.Identity,
                            bias=bias(2, m)[hh * 64:hh * 64 + 64, :], scale=1.0,
                        )
                    else:
                        nc.vector.tensor_scalar_add(
                            xaT[hh * 64:hh * 64 + 64, m * SEQ + c * 128: m * SEQ + (c + 1) * 128],
                            pvh[:], bias(2, m)[hh * 64:hh * 64 + 64, :])
            for hh in range(2):
                nc.sync.dma_start(
                    attn_d[2 * hp + hh].rearrange("c r m -> r c m"),
                    attb[:].rearrange("p (c h m) -> p c h m", c=NCH, h=2)[:, :, hh, :],
                )

        # -------- emission schedule: feed PE projections early, attention follows
        phase = globals().get("_PHASE", "full")
        if phase != "dma":
            if phase == "proj":
                for m in range(G):
                    proj_q(m); proj_k(m)
                for tb in range(NTB):
                    proj_v(tb)
            else:
                proj_q(0); proj_k(0)
                attn_pair(0, fillers=[
                    [lambda: proj_v(0), lambda: proj_v(1)],
                    [lambda: proj_v(2)],
                    [lambda: proj_v(3)],
                    [lambda: proj_v(4)],
                ])
                npair = {"attn2": 2}.get(phase, G)
                if npair > 1:
                    proj_q(1); proj_k(1)
                def oproj_half1(c_lo=0, c_hi=NCH):
                    if phase != "full":
                        return
                    for c in range(c_lo, c_hi):
                        for n in range(2):
                            po = ppool.tile([128, 512], F32, tag="pp",
                                            name=f"po1_{c}_{n}")
                            for g in range(4):
                                nc.tensor.matmul(
                                    po[:],
                                    xaT[:, g * SEQ + c * 128: g * SEQ + (c + 1) * 128],
                                    wo_t[:, g * D + n * 512: g * D + (n + 1) * 512],
                                    start=(g == 0), stop=(g == 3),
                                )
                            nc.vector.tensor_copy(
                                ob1_t[:, (c * 2 + n) * 512:(c * 2 + n + 1) * 512], po[:])

                for m in range(1, npair):
                    nm = m + 1
                    if nm < npair:
                        fl = [[lambda nm=nm: proj_q(nm)], [lambda nm=nm: proj_k(nm)]]
                    else:
                        fl = []
                    attn_pair(m, fillers=fl)
                    ksplit = os.environ.get("K_OPJ", "6")
                    if ksplit == "56":
                        if m == 5:
                            oproj_half1(0, 2)
                        elif m == 6:
                            oproj_half1(2, 4)
                    else:
                        if m == 6:
                            oproj_half1()

        # -------- output projection, second half (g=4..7) + add of first half
        for c in range(NCH if phase == "full" else 0):
            ob = work.tile([128, D], F32, tag="ob")
            for n in range(2):
                po = ppool.tile([128, 512], F32, tag="pp")
                for g in range(4, G):
                    nc.tensor.matmul(
                        po[:],
                        xaT[:, g * SEQ + c * 128: g * SEQ + (c + 1) * 128],
                        wo_t[:, g * D + n * 512: g * D + (n + 1) * 512],
                        start=(g == 4), stop=(g == G - 1),
                    )
                nc.vector.scalar_tensor_tensor(
                    ob[:, n * 512:(n + 1) * 512], po[:], 0.0,
                    ob1_t[:, (c * 2 + n) * 512:(c * 2 + n + 1) * 512],
                    op0=ALU.add, op1=ALU.add,
                )
                nc.sync.dma_start(
                    out_d[c * 128:(c + 1) * 128, n * 512:(n + 1) * 512],
                    ob[:, n * 512:(n + 1) * 512])

    nc.compile()
    return nc


def _shard_inputs(query, key, value, Wq, bq, Wk, bk, Wv, bv, Wo):
    bf = ml_dtypes.bfloat16
    wq = np.asarray(Wq, np.float32).astype(bf)
    wk = np.asarray(Wk, np.float32).astype(bf)
    wv = np.asarray(Wv, np.float32).astype(bf)
    wo = np.asarray(Wo, np.float32).astype(bf)
    b3 = np.stack([np.asarray(bq, np.float32), np.asarray(bk, np.float32),
                   np.asarray(bv, np.float32)])
    ident = np.eye(128, dtype=bf)

    r_ = np.arange(128)[:, None]
    m_ = np.arange(256)[None, :]
    band = (m_ - r_ >= 0) & (m_ - r_ <= 2 * HALO)

    in_maps = []
    for core in range(8):
        b, blk = divmod(core, NBLK)
        q0 = blk * SEQ
        lo, hi = q0 - HALO, q0 + SEQ + HALO
        s0, s1 = max(lo, 0), min(hi, S)
        xq = np.asarray(query[b, q0:q0 + SEQ, :], np.float32).T.astype(bf)
        xk = np.zeros((D, KV), bf)
        xv = np.zeros((D, KV), bf)
        xk[:, s0 - lo:s1 - lo] = np.asarray(key[b, s0:s1, :], np.float32).T.astype(bf)
        xv[:, s0 - lo:s1 - lo] = np.asarray(value[b, s0:s1, :], np.float32).T.astype(bf)
        mask = np.zeros((NCH, 128, 256), np.float32)
        for c in range(NCH):
            j = lo + c * 128 + m_
            mask[c] = (band & (j >= 0) & (j < S)).astype(np.float32)
        mask = mask.astype(bf)
        in_maps.append({
            "xqT": xq, "xkT": xk, "xvT": xv,
            "Wq": wq, "Wk": wk, "Wv": wv, "Wo": wo,
            "b3": b3, "mask": mask, "ident": ident,
        })
    return in_maps


def kernel(query, key, value, Wq, bq, Wk, bk, Wv, bv, Wo, bo):
    global _NC, LAST_RESULTS
    if _NC is None:
        _NC = _build()
    in_maps = _shard_inputs(query, key, value, Wq, bq, Wk, bk, Wv, bv, Wo)
    res = bass_utils.run_bass_kernel_spmd(_NC, in_maps, core_ids=list(range(8)))
    LAST_RESULTS = res

    bo32 = np.asarray(bo, np.float32)
    out = np.empty((B, S, D), np.float32)
    attn = np.zeros((B, H, S, S), np.float32)
    for core in range(8):
        rr = res.results[core]
        b, blk = divmod(core, NBLK)
        q0 = blk * SEQ
        out[b, q0:q0 + SEQ, :] = rr["out_d"] + bo32[None, :]
        ac = np.asarray(rr["attn_c"]).astype(np.float32)  # [H, NCH, 128, 256]
        lo = q0 - HALO
        for c in range(NCH):
            j0 = lo + c * 128
            jlo, jhi = max(j0, 0), min(j0 + 256, S)
            attn[b, :, q0 + c * 128: q0 + (c + 1) * 128, jlo:jhi] = \
                ac[:, c, :, jlo - j0:jhi - j0]
    return out, attn

